# revision 6
# baseline (speedup 1.0000x reference)
"""GCN edge-decoder kernel for Trainium2 (8 NeuronCores, SPMD).

out[e] = W2 . relu(x[src[e]] @ W1[:128] + x[dst[e]] @ W1[128:] + b1) + b2

Strategy (edge-parallel, per sharding hint):
- Shard the 640k edges across 8 cores (80k each); replicate x and weights.
- Host prep: cast x/W1/W2 to bf16 (halves gather bytes, full-rate PE),
  split W1 into src/dst halves, transpose per-core edge indices into
  [128, 625] int32 tiles (one index per partition per gather).
- Device per 512-edge chunk: indirect-DMA gather of x rows (128 rows/DMA),
  PE transpose to feature-major, two accumulating matmuls with the W1
  halves, fused bias+ReLU on the scalar engine, then a [128,1] matmul
  with W2 and one small DMA out.
"""

import sys

for _p in ("/opt/trn_rl_repo", "/root/.axon_site/_ro/trn_rl_repo"):
    if _p not in sys.path:
        sys.path.insert(0, _p)

import numpy as np
import ml_dtypes

N, D, E = 100000, 128, 640000
NCORES = 8
EPC = E // NCORES  # 80000 edges per core
TILES = EPC // 128  # 625 gather tiles of 128 edges
SUB = 4  # 128-edge subtiles per chunk (512-edge chunks)


def _build():
    from concourse import bacc, bass, mybir
    import concourse.tile as tile
    from concourse.masks import make_identity

    dt = mybir.dt
    nc = bacc.Bacc(
        "TRN2", target_bir_lowering=False, debug=True,
        enable_asserts=False, num_devices=NCORES,
    )
    x_d = nc.dram_tensor("x_bf16", [N, D], dt.bfloat16, kind="ExternalInput").ap()
    w1s_d = nc.dram_tensor("w1s", [D, D], dt.bfloat16, kind="ExternalInput").ap()
    w1d_d = nc.dram_tensor("w1d", [D, D], dt.bfloat16, kind="ExternalInput").ap()
    b1_d = nc.dram_tensor("b1", [D, 1], dt.float32, kind="ExternalInput").ap()
    w2_d = nc.dram_tensor("w2", [D, 1], dt.bfloat16, kind="ExternalInput").ap()
    b2_d = nc.dram_tensor("b2", [1, 1], dt.float32, kind="ExternalInput").ap()
    si_d = nc.dram_tensor("src_idx", [128, TILES], dt.int32, kind="ExternalInput").ap()
    di_d = nc.dram_tensor("dst_idx", [128, TILES], dt.int32, kind="ExternalInput").ap()
    out_d = nc.dram_tensor("out", [1, EPC], dt.float32, kind="ExternalOutput").ap()

    W = SUB * 128

    with tile.TileContext(nc) as tc:
        with tc.tile_pool(name="const", bufs=1) as cp, \
             tc.tile_pool(name="io", bufs=3) as io, \
             tc.tile_pool(name="trs", bufs=2) as trs, \
             tc.tile_pool(name="ps_tr", bufs=2, space="PSUM") as ps_tr, \
             tc.tile_pool(name="ps_h", bufs=2, space="PSUM") as ps_h, \
             tc.tile_pool(name="ps_o", bufs=2, space="PSUM") as ps_o:
            ident = cp.tile([128, 128], dt.bfloat16)
            make_identity(nc, ident[:])
            w1s = cp.tile([D, D], dt.bfloat16)
            nc.sync.dma_start(w1s[:], w1s_d[:])
            w1d = cp.tile([D, D], dt.bfloat16)
            nc.sync.dma_start(w1d[:], w1d_d[:])
            b1 = cp.tile([D, 1], dt.float32)
            nc.sync.dma_start(b1[:], b1_d[:])
            w2 = cp.tile([D, 1], dt.bfloat16)
            nc.sync.dma_start(w2[:], w2_d[:])
            b2s = cp.tile([1, 1], dt.float32)
            nc.sync.dma_start(b2s[:], b2_d[:])
            si = cp.tile([128, TILES], dt.int32)
            nc.sync.dma_start(si[:], si_d[:])
            di = cp.tile([128, TILES], dt.int32)
            nc.sync.dma_start(di[:], di_d[:])

            for t0 in range(0, TILES, SUB):
                ns = min(SUB, TILES - t0)
                w = ns * 128
                A = io.tile([128, W], dt.bfloat16, tag="A")
                B = io.tile([128, W], dt.bfloat16, tag="B")
                for s in range(ns):
                    nc.gpsimd.indirect_dma_start(
                        out=A[:, s * 128:(s + 1) * 128], out_offset=None,
                        in_=x_d[:],
                        in_offset=bass.IndirectOffsetOnAxis(
                            ap=si[:, t0 + s:t0 + s + 1], axis=0),
                    )
                    nc.gpsimd.indirect_dma_start(
                        out=B[:, s * 128:(s + 1) * 128], out_offset=None,
                        in_=x_d[:],
                        in_offset=bass.IndirectOffsetOnAxis(
                            ap=di[:, t0 + s:t0 + s + 1], axis=0),
                    )
                At_p = ps_tr.tile([128, W], dt.bfloat16, tag="Atp")
                Bt_p = ps_tr.tile([128, W], dt.bfloat16, tag="Btp")
                for s in range(ns):
                    sl = slice(s * 128, (s + 1) * 128)
                    nc.tensor.transpose(At_p[:, sl], A[:, sl], ident[:])
                    nc.tensor.transpose(Bt_p[:, sl], B[:, sl], ident[:])
                At = trs.tile([128, W], dt.bfloat16, tag="At")
                Bt = trs.tile([128, W], dt.bfloat16, tag="Bt")
                nc.vector.tensor_copy(At[:, :w], At_p[:, :w])
                nc.vector.tensor_copy(Bt[:, :w], Bt_p[:, :w])
                h_p = ps_h.tile([128, W], dt.float32, tag="h")
                nc.tensor.matmul(h_p[:, :w], lhsT=w1s[:], rhs=At[:, :w],
                                 start=True, stop=False)
                nc.tensor.matmul(h_p[:, :w], lhsT=w1d[:], rhs=Bt[:, :w],
                                 start=False, stop=True)
                R = trs.tile([128, W], dt.bfloat16, tag="R")
                nc.scalar.activation(R[:, :w], h_p[:, :w],
                                     mybir.ActivationFunctionType.Relu,
                                     bias=b1[:])
                o_p = ps_o.tile([1, W], dt.float32, tag="o")
                nc.tensor.matmul(o_p[:1, :w], lhsT=w2[:], rhs=R[:, :w],
                                 start=True, stop=True)
                o_sb = io.tile([1, W], dt.float32, tag="osb")
                nc.vector.tensor_scalar_add(o_sb[:1, :w], o_p[:1, :w],
                                            b2s[:1, :1])
                nc.sync.dma_start(out_d[:1, t0 * 128:t0 * 128 + w],
                                  o_sb[:1, :w])
    nc.compile()
    return nc


def kernel(x, edge_index, W1, b1, W2, b2, _trace=False):
    from concourse import bass_utils

    x = np.asarray(x)
    edge_index = np.asarray(edge_index)
    W1 = np.asarray(W1)
    b1 = np.asarray(b1)
    W2 = np.asarray(W2)
    b2 = np.asarray(b2)

    bf16 = ml_dtypes.bfloat16
    x_bf = x.astype(bf16)
    w1s = np.ascontiguousarray(W1[:D, :]).astype(bf16)
    w1d = np.ascontiguousarray(W1[D:, :]).astype(bf16)
    b1_c = np.ascontiguousarray(b1.astype(np.float32).reshape(D, 1))
    w2_c = np.ascontiguousarray(W2.astype(np.float32).reshape(D, 1)).astype(bf16)
    b2_c = np.ascontiguousarray(b2.astype(np.float32).reshape(1, 1))

    src = edge_index[0].astype(np.int32)
    dst = edge_index[1].astype(np.int32)

    in_maps = []
    for c in range(NCORES):
        sl = slice(c * EPC, (c + 1) * EPC)
        si = np.ascontiguousarray(src[sl].reshape(TILES, 128).T)
        di = np.ascontiguousarray(dst[sl].reshape(TILES, 128).T)
        in_maps.append({
            "x_bf16": x_bf, "w1s": w1s, "w1d": w1d, "b1": b1_c,
            "w2": w2_c, "b2": b2_c, "src_idx": si, "dst_idx": di,
        })

    nc = _build()
    res = bass_utils.run_bass_kernel_spmd(
        nc, in_maps, core_ids=list(range(NCORES)), trace=_trace,
    )
    outs = [np.asarray(r["out"]).reshape(-1) for r in res.results]
    full = np.concatenate(outs).astype(np.float32)
    if _trace:
        kernel._last_exec_time_ns = res.exec_time_ns
        kernel._last_profile = res.profile_json
    return full
